# revision 16
# baseline (speedup 1.0000x reference)
"""Trainium2 Bass kernel for nn_DRNN_75204877353433 (v2).

Windowed bidirectional GRU (W=15) over [B=32, S=512] token ids ->
batch-norm (training stats over B,S) -> MLP -> masked max-pool -> linear.

v2 strategy (8 NeuronCores, data-parallel over batch; BC=4 rows/core):
  - ACT (scalar) engine is the steady-state bottleneck (3 sigmoid/tanh
    elements per gate-dim per step, 1 elem/cycle/lane, no fast mode), so
    the scan is restructured so every other engine stays under ACT's
    ~3.2us/iteration period:
      * recurrent matmuls h @ W_hh run in fp8e4 with DoubleRow perf mode
        (contraction 256 in one pass, 0.5 cycles/row): h keeps an fp16
        master copy (accuracy) and the Pool engine writes an fp8 mirror
        used only as matmul input;
      * xr/xz injections into PSUM stay on the PE as identity matmuls
        (cheapest engine for a +tile into PSUM);
      * sigmoids are merged per k-tile ([r|z] in one 2-bank PSUM tile),
        tanh merged across k;
      * elementwise chain is split DVE/Pool so neither exceeds ACT.
  - program order is stage-interleaved across the 8 (c,d) streams per
    window step so every engine sees a homogeneous run of ready work
    instead of serialized per-stream chains (the v1 bottleneck).
"""

import sys

for _p in ("/opt/trn_rl_repo",):
    if _p not in sys.path:
        sys.path.insert(0, _p)

import numpy as np

from concourse import bacc, mybir, tile
from concourse.bass import IndirectOffsetOnAxis
from concourse.bass_utils import run_bass_kernel_spmd

F32 = mybir.dt.float32
BF16 = mybir.dt.float16    # 16-bit compute dtype: fp16 (10-bit mantissa)
FP8 = mybir.dt.float8e4
I32 = mybir.dt.int32
AF = mybir.ActivationFunctionType
OP = mybir.AluOpType
AX = mybir.AxisListType
DR = mybir.MatmulPerfMode.DoubleRow


class Cfg:
    def __init__(self, B=32, S=512, W=15, E=300, H=256, C=2, n_cores=8,
                 use_cc=True, use_fp8=True, repeat=1,
                 prz_bufs=2, pn_bufs=2, gate_bufs=4, group=3):
        self.B, self.S, self.W, self.E, self.H, self.C = B, S, W, E, H, C
        self.n_cores = n_cores
        self.use_cc = use_cc
        self.use_fp8 = use_fp8
        self.repeat = repeat
        self.prz_bufs = prz_bufs
        self.pn_bufs = pn_bufs
        self.gate_bufs = gate_bufs
        self.group = group
        self.G = 3 * H
        self.BC = B // n_cores                      # batch rows per core
        seg = S + 2 * (W - 1)                       # valid token cols per row
        self.SEG = seg
        while (self.BC * self.SEG) % 128:
            self.SEG += 1
        self.TC = self.BC * self.SEG                # token cols per core
        self.NT = self.TC // 128                    # gather tiles
        self.NR = self.BC * S                       # window rows per core
        self.HK = (H + 127) // 128                  # H partition tiles (2)
        self.GS = self.G // 128                     # G subtiles (6)
        self.EK = [(k * 128, min(128, E - k * 128))
                   for k in range((E + 127) // 128)]
        self.CHT = (2 * H) // 128                   # hidden channel tiles (4)
        self.XCH = [(i * 512, min(512, self.TC - i * 512))
                    for i in range((self.TC + 511) // 512)]
        assert H % 128 == 0 and self.G % 128 == 0 and self.HK == 2


def build(cfg: Cfg):
    """Build + bacc-compile the Bass program. Returns nc."""
    nc = bacc.Bacc("TRN2", target_bir_lowering=False, debug=False,
                   enable_asserts=False, num_devices=cfg.n_cores)
    _eps_t = nc.alloc_sbuf_tensor("const-eps", [128, 1], F32)
    nc.gpsimd.memset(_eps_t.ap(), 1e-5)
    nc.const_aps.aps[(F32, 1e-5)] = _eps_t.ap()
    nc.all_engine_barrier()
    B, S, W, E, H, C = cfg.B, cfg.S, cfg.W, cfg.E, cfg.H, cfg.C
    BC, SEG, TC, NT, NR, HK, GS = (cfg.BC, cfg.SEG, cfg.TC, cfg.NT, cfg.NR,
                                   cfg.HK, cfg.GS)

    def din(name, shape, dt):
        return nc.dram_tensor(name, shape, dt, kind="ExternalInput").ap()

    ptab = din("ptab", [TC, E], BF16)
    ids = din("ids", [TC, 1], I32)
    maskin = din("maskin", [128, NR], BF16)
    wih = [din(f"wih{d}", [E, cfg.G], BF16) for d in range(2)]
    whh2 = [din(f"whh2{d}", [128, 2 * cfg.G], BF16) for d in range(2)]
    bgd = [din(f"bg{d}", [128, GS], F32) for d in range(2)]
    bhnd = [din(f"bhn{d}", [128, HK], F32) for d in range(2)]
    identf_d = din("identf", [128, 128], F32)
    identb_d = din("identb", [128, 128], BF16)
    bng_d = din("bng", [128, cfg.CHT], F32)
    bnb_d = din("bnb", [128, cfg.CHT], F32)
    mb65_d = din("mb65", [128, cfg.CHT], F32)
    mwt_d = din("mwt", [2 * H, 2 * H], BF16)
    lwt_d = din("lwt", [2 * H, C], F32)
    lb4_d = din("lb4", [BC, C], F32)
    out_d = nc.dram_tensor("out", [BC, C], F32, kind="ExternalOutput").ap()

    inv_n = 1.0 / float(B * S)

    with tile.TileContext(nc) as tc:
        # ---- persistent constants -------------------------------------
        constp = tc.alloc_tile_pool(name="const", bufs=1)
        identf = constp.tile([128, 128], F32)
        identb = constp.tile([128, 128], BF16)
        nc.sync.dma_start(identf[:], identf_d[:])
        nc.sync.dma_start(identb[:], identb_d[:])
        whh16 = [constp.tile([128, 2, cfg.G], BF16, name=f"whh16_{d}")
                 for d in range(2)]
        for d in range(2):
            nc.sync.dma_start(whh16[d][:], whh2[d][:].rearrange(
                "p (j g) -> p j g", j=2))
        if cfg.use_fp8:
            whh8 = [constp.tile([128, 2, cfg.G], FP8, name=f"whh8_{d}")
                    for d in range(2)]
            for d in range(2):
                nc.vector.tensor_copy(out=whh8[d][:], in_=whh16[d][:])
        bg_t = [constp.tile([128, GS], F32, name=f"bg{d}") for d in range(2)]
        bhn_t = [constp.tile([128, HK], F32, name=f"bhn{d}") for d in range(2)]
        for d in range(2):
            nc.sync.dma_start(bg_t[d][:], bgd[d][:])
            nc.sync.dma_start(bhn_t[d][:], bhnd[d][:])

        # persistent state tiles
        xgp = tc.alloc_tile_pool(name="xg", bufs=1)
        xg = [xgp.tile([128, GS, TC], BF16, name=f"xg{d}") for d in range(2)]
        hp = tc.alloc_tile_pool(name="h", bufs=1)
        h_m = [hp.tile([128, 2, NR], BF16, name=f"hm{d}") for d in range(2)]
        if cfg.use_fp8:
            h8p = tc.alloc_tile_pool(name="h8", bufs=1)
            h_8 = [h8p.tile([128, 2, NR], FP8, name=f"h8{d}") for d in range(2)]
        hidp = tc.alloc_tile_pool(name="hid", bufs=1, side="right")
        hid = hidp.tile([128, cfg.CHT, NR], BF16)
        maskp = tc.alloc_tile_pool(name="maskp", bufs=1, side="right")
        mask_t = maskp.tile([128, NR], BF16)
        moff_t = maskp.tile([128, NR], F32)

        from contextlib import nullcontext
        rep_ctx = tc.For_i(0, cfg.repeat, 1) if cfg.repeat > 1 \
            else nullcontext()
        rep_ctx.__enter__()

        nc.sync.dma_start(mask_t[:], maskin[:])
        # moff = (mask-1)*65500 : 0 live, -65500 dead (phase D additive)
        nc.vector.tensor_scalar(
            out=moff_t[:], in0=mask_t[:], scalar1=1.0, scalar2=65500.0,
            op0=OP.subtract, op1=OP.mult)

        # ---- phase A: gather + transpose + xg precompute ---------------
        with tc.tile_pool(name="wihp", bufs=1) as wihp, \
             tc.tile_pool(name="idsp", bufs=2) as idsp, \
             tc.tile_pool(name="eraw", bufs=3) as erawp, \
             tc.tile_pool(name="eT", bufs=1) as eTp, \
             tc.tile_pool(name="tpsum", bufs=2, space="PSUM") as tpsump, \
             tc.tile_pool(name="xgpsum", bufs=4, space="PSUM") as xgpsump:
            wih_t = [[wihp.tile([128, cfg.G], BF16, name=f"wih{d}_{k}")
                      for k in range(len(cfg.EK))] for d in range(2)]
            for d in range(2):
                for k, (e0, ew) in enumerate(cfg.EK):
                    nc.sync.dma_start(wih_t[d][k][:ew, :], wih[d][e0:e0 + ew, :])
            eT = eTp.tile([128, len(cfg.EK), TC], BF16)
            for t in range(NT):
                idt = idsp.tile([128, 1], I32)
                nc.sync.dma_start(idt[:], ids[t * 128:(t + 1) * 128, :])
                er = erawp.tile([128, E], BF16)
                nc.gpsimd.indirect_dma_start(
                    out=er[:], out_offset=None, in_=ptab[:],
                    in_offset=IndirectOffsetOnAxis(ap=idt[:, :1], axis=0),
                )
                tp = tpsump.tile([128, len(cfg.EK), 128], BF16, space="PSUM")
                for k, (e0, ew) in enumerate(cfg.EK):
                    nc.tensor.transpose(out=tp[:ew, k, :],
                                        in_=er[:, e0:e0 + ew],
                                        identity=identb[:])
                nc.vector.tensor_copy(
                    out=eT[:, :, t * 128:(t + 1) * 128], in_=tp[:])
            nkk = len(cfg.EK)
            cnt = 0
            for d in range(2):
                for g in range(GS):
                    for (c0, cw) in cfg.XCH:
                        p = xgpsump.tile([128, 512], F32, space="PSUM")
                        for k, (e0, ew) in enumerate(cfg.EK):
                            nc.tensor.matmul(
                                p[:, :cw],
                                lhsT=wih_t[d][k][:ew, g * 128:(g + 1) * 128],
                                rhs=eT[:ew, k, c0:c0 + cw],
                                start=(k == 0), stop=(k == nkk - 1))
                        dst = xg[d][:, g, c0:c0 + cw]
                        if cnt % 3 == 2:
                            nc.vector.tensor_single_scalar(
                                out=dst, in_=p[:, :cw],
                                scalar=bg_t[d][:, g:g + 1], op=OP.add)
                        else:
                            nc.scalar.activation(
                                out=dst, in_=p[:, :cw],
                                func=AF.Identity, bias=bg_t[d][:, g:g + 1])
                        cnt += 1

        # ---- phase B: the windowed GRU scan ----------------------------
        # streams: (c, d); g-tile order in xg/whh: [r0, r1, z0, z1, n0, n1]
        streams = [(c, d) for c in range(BC) for d in range(2)]

        def base_of(c, d, w):
            off = w if d == 0 else 2 * (W - 1) - w
            return c * SEG + off

        GB = cfg.gate_bufs
        with tc.tile_pool(name="prz", bufs=cfg.prz_bufs, space="PSUM") as przp, \
             tc.tile_pool(name="pn", bufs=cfg.pn_bufs, space="PSUM") as pnp, \
             tc.tile_pool(name="rz4", bufs=GB) as rzp, \
             tc.tile_pool(name="tsb", bufs=GB) as tsp, \
             tc.tile_pool(name="tnb", bufs=GB) as tnp, \
             tc.tile_pool(name="nb", bufs=GB) as nbp, \
             tc.tile_pool(name="db", bufs=GB) as dbp, \
             tc.tile_pool(name="eb", bufs=GB) as ebp:

            groups = [streams[i:i + cfg.group]
                      for i in range(0, len(streams), cfg.group)]

            def emit_w0(grp):
                """h1 = (1-z)*n, n = tanh(xn + r*bhn); h0 = 0."""
                rzs, ts, tns, ns = {}, {}, {}, {}
                for s in grp:
                    c, d = s
                    b0 = base_of(c, d, 0)
                    rz4 = rzp.tile([128, 4, S], BF16, tag="rz")
                    # blocks [rk0, zk0, rk1, zk1]; xg g-order [r0,r1,z0,z1]
                    for k in range(2):
                        nc.scalar.activation(
                            out=rz4[:, 2 * k, :], in_=xg[d][:, k, b0:b0 + S],
                            func=AF.Sigmoid)
                        nc.scalar.activation(
                            out=rz4[:, 2 * k + 1, :],
                            in_=xg[d][:, 2 + k, b0:b0 + S], func=AF.Sigmoid)
                    rzs[s] = rz4
                for s in grp:
                    c, d = s
                    t_ = tsp.tile([128, 2, S], BF16, tag="t")
                    for k in range(2):
                        nc.vector.tensor_single_scalar(
                            out=t_[:, k, :], in_=rzs[s][:, 2 * k, :],
                            scalar=bhn_t[d][:, k:k + 1], op=OP.mult)
                    ts[s] = t_
                for s in grp:
                    c, d = s
                    b0 = base_of(c, d, 0)
                    tn_ = tnp.tile([128, 2, S], BF16, tag="tn")
                    nc.gpsimd.tensor_tensor(
                        out=tn_[:], in0=ts[s][:], in1=xg[d][:, 4:6, b0:b0 + S],
                        op=OP.add)
                    tns[s] = tn_
                for s in grp:
                    n_ = nbp.tile([128, 2, S], BF16, tag="n")
                    nc.scalar.activation(out=n_[:], in_=tns[s][:], func=AF.Tanh)
                    ns[s] = n_
                for s in grp:
                    c, d = s
                    hc = slice(c * S, (c + 1) * S)
                    nc.vector.scalar_tensor_tensor(
                        out=h_m[d][:, :, hc], in0=rzs[s][:, 1::2, :],
                        scalar=1.0, in1=ns[s][:],
                        op0=OP.subtract, op1=OP.mult)
                if cfg.use_fp8:
                    for s in grp:
                        c, d = s
                        hc = slice(c * S, (c + 1) * S)
                        nc.gpsimd.tensor_copy(out=h_8[d][:, :, hc],
                                              in_=h_m[d][:, :, hc])

            def emit_w(w, grp):
                last = (w == W - 1)
                przs, pns, rzs, ts, tns, ns, ds, es = ({}, {}, {}, {}, {},
                                                       {}, {}, {})
                for s in grp:
                    c, d = s
                    b0 = base_of(c, d, w)
                    hc = slice(c * S, (c + 1) * S)
                    prz = [przp.tile([128, 2, S], F32, space="PSUM",
                                     tag="prz", name=f"prz{k}")
                           for k in range(2)]
                    pn = pnp.tile([128, 2, S], F32, space="PSUM", tag="pn")
                    # xr/xz injections (identity matmuls, start=True)
                    for k in range(2):
                        nc.tensor.matmul(prz[k][:, 0, :], lhsT=identb[:],
                                         rhs=xg[d][:, k, b0:b0 + S],
                                         start=True, stop=False)
                        nc.tensor.matmul(prz[k][:, 1, :], lhsT=identb[:],
                                         rhs=xg[d][:, 2 + k, b0:b0 + S],
                                         start=True, stop=False)
                    # recurrent matmuls
                    if cfg.use_fp8:
                        rhs8 = h_8[d][:, :, hc]
                        for k in range(2):
                            nc.tensor.matmul(
                                prz[k][:, 0, :],
                                lhsT=whh8[d][:, :, k * 128:(k + 1) * 128],
                                rhs=rhs8, start=False, stop=True, perf_mode=DR)
                            nc.tensor.matmul(
                                prz[k][:, 1, :],
                                lhsT=whh8[d][:, :, 256 + k * 128:256 + (k + 1) * 128],
                                rhs=rhs8, start=False, stop=True, perf_mode=DR)
                            nc.tensor.matmul(
                                pn[:, k, :],
                                lhsT=whh8[d][:, :, 512 + k * 128:512 + (k + 1) * 128],
                                rhs=rhs8, start=True, stop=True, perf_mode=DR)
                    else:
                        for k in range(2):
                            for kk in range(2):
                                lw = whh16[d][:, kk, :]
                                rh = h_m[d][:, kk, hc]
                                nc.tensor.matmul(
                                    prz[k][:, 0, :],
                                    lhsT=lw[:, k * 128:(k + 1) * 128],
                                    rhs=rh, start=False, stop=(kk == 1))
                                nc.tensor.matmul(
                                    prz[k][:, 1, :],
                                    lhsT=lw[:, 256 + k * 128:256 + (k + 1) * 128],
                                    rhs=rh, start=False, stop=(kk == 1))
                                nc.tensor.matmul(
                                    pn[:, k, :],
                                    lhsT=lw[:, 512 + k * 128:512 + (k + 1) * 128],
                                    rhs=rh, start=(kk == 0), stop=(kk == 1))
                    przs[s], pns[s] = prz, pn
                for s in grp:
                    rz4 = rzp.tile([128, 4, S], BF16, tag="rz")
                    for k in range(2):
                        nc.scalar.activation(
                            out=rz4[:, 2 * k:2 * k + 2, :],
                            in_=przs[s][k][:], func=AF.Sigmoid)
                    rzs[s] = rz4
                for s in grp:
                    c, d = s
                    t_ = tsp.tile([128, 2, S], BF16, tag="t")
                    for k in range(2):
                        nc.vector.scalar_tensor_tensor(
                            out=t_[:, k, :], in0=pns[s][:, k, :],
                            scalar=bhn_t[d][:, k:k + 1],
                            in1=rzs[s][:, 2 * k, :], op0=OP.add, op1=OP.mult)
                    ts[s] = t_
                for s in grp:
                    c, d = s
                    b0 = base_of(c, d, w)
                    tn_ = tnp.tile([128, 2, S], BF16, tag="tn")
                    nc.gpsimd.tensor_tensor(
                        out=tn_[:], in0=ts[s][:],
                        in1=xg[d][:, 4:6, b0:b0 + S], op=OP.add)
                    tns[s] = tn_
                for s in grp:
                    n_ = nbp.tile([128, 2, S], BF16, tag="n")
                    nc.scalar.activation(out=n_[:], in_=tns[s][:], func=AF.Tanh)
                    ns[s] = n_
                for s in grp:
                    c, d = s
                    hc = slice(c * S, (c + 1) * S)
                    d_ = dbp.tile([128, 2, S], BF16, tag="d")
                    nc.gpsimd.tensor_tensor(
                        out=d_[:], in0=h_m[d][:, :, hc], in1=ns[s][:],
                        op=OP.subtract)
                    ds[s] = d_
                for s in grp:
                    e_ = ebp.tile([128, 2, S], BF16, tag="e")
                    nc.vector.tensor_tensor(
                        out=e_[:], in0=rzs[s][:, 1::2, :], in1=ds[s][:],
                        op=OP.mult)
                    es[s] = e_
                for s in grp:
                    c, d = s
                    hc = slice(c * S, (c + 1) * S)
                    dest = hid[:, 2 * d:2 * d + 2, hc] if last \
                        else h_m[d][:, :, hc]
                    nc.vector.tensor_tensor(
                        out=dest, in0=ns[s][:], in1=es[s][:], op=OP.add)
                if cfg.use_fp8 and not last:
                    for s in grp:
                        c, d = s
                        hc = slice(c * S, (c + 1) * S)
                        nc.gpsimd.tensor_copy(out=h_8[d][:, :, hc],
                                              in_=h_m[d][:, :, hc])

            for grp in groups:
                emit_w0(grp)
            for w in range(1, W):
                for grp in groups:
                    emit_w(w, grp)

        if cfg.repeat == 1:
            if cfg.use_fp8:
                h8p.release()
            hp.release()
            xgp.release()

        # ---- phase C: BN stats + AllReduce + affine --------------------
        nrmp = tc.alloc_tile_pool(name="nrm", bufs=1, side="right")
        nrm = nrmp.tile([128, cfg.CHT, NR], BF16)
        with tc.tile_pool(name="scr", bufs=2) as scrp, \
             tc.tile_pool(name="stat", bufs=1) as statp, \
             tc.tile_pool(name="dram", bufs=1, space="DRAM") as dramp:
            sums = statp.tile([128, 2 * cfg.CHT], F32)
            dummy = statp.tile([128, 1], F32)
            for ct in range(cfg.CHT):
                sc = scrp.tile([128, NR], BF16, tag="scr")
                nc.vector.tensor_tensor(out=sc[:], in0=hid[:, ct, :],
                                        in1=mask_t[:], op=OP.mult)
                nc.vector.tensor_reduce(out=sums[:, ct:ct + 1],
                                        in_=sc[:], axis=AX.X, op=OP.add)
                sq = scrp.tile([128, NR], BF16, tag="scr2")
                nc.vector.tensor_tensor(out=sq[:], in0=sc[:], in1=sc[:],
                                        op=OP.mult)
                nc.vector.tensor_reduce(
                    out=sums[:, cfg.CHT + ct:cfg.CHT + ct + 1],
                    in_=sq[:], axis=AX.X, op=OP.add)
            gsums = statp.tile([128, 2 * cfg.CHT], F32)
            if cfg.use_cc:
                bnc_in = dramp.tile([128, 2 * cfg.CHT], F32)
                bnc_out = dramp.tile([128, 2 * cfg.CHT], F32,
                                     addr_space="Shared")
                nc.gpsimd.dma_start(bnc_in[:], sums[:])
                nc.gpsimd.collective_compute(
                    "AllReduce", OP.add,
                    replica_groups=[list(range(cfg.n_cores))],
                    ins=[bnc_in.opt()], outs=[bnc_out.opt()])
                nc.gpsimd.dma_start(gsums[:], bnc_out[:])
            else:
                nc.vector.tensor_copy(out=gsums[:], in_=sums[:])

            bng_t = statp.tile([128, cfg.CHT], F32)
            bnb_t = statp.tile([128, cfg.CHT], F32)
            nc.sync.dma_start(bng_t[:], bng_d[:])
            nc.sync.dma_start(bnb_t[:], bnb_d[:])
            abuf = statp.tile([128, cfg.CHT], F32)
            bbuf = statp.tile([128, cfg.CHT], F32)
            with nc.allow_low_precision("bn 1/sqrt + NR refine"), \
                 tc.tile_pool(name="stt", bufs=2) as sttp:
                for ct in range(cfg.CHT):
                    gs_s = gsums[:, ct:ct + 1]
                    gs_q = gsums[:, cfg.CHT + ct:cfg.CHT + ct + 1]
                    mu = sttp.tile([128, 1], F32, tag="mu")
                    nc.scalar.mul(mu[:], gs_s, inv_n)
                    mq = sttp.tile([128, 1], F32, tag="mq")
                    nc.scalar.square(mq[:], mu[:])
                    varp = sttp.tile([128, 1], F32, tag="var")
                    nc.vector.scalar_tensor_tensor(
                        out=varp[:], in0=gs_q, scalar=inv_n, in1=mq[:],
                        op0=OP.mult, op1=OP.subtract)
                    nc.scalar.add(varp[:], varp[:], 1e-5)
                    sd = sttp.tile([128, 1], F32, tag="sd")
                    nc.scalar.sqrt(sd[:], varp[:])
                    y0 = sttp.tile([128, 1], F32, tag="y0")
                    nc.vector.reciprocal(y0[:], sd[:])
                    y2 = sttp.tile([128, 1], F32, tag="y2")
                    nc.vector.tensor_tensor(out=y2[:], in0=y0[:], in1=y0[:],
                                            op=OP.mult)
                    vy2 = sttp.tile([128, 1], F32, tag="vy2")
                    nc.vector.tensor_tensor(out=vy2[:], in0=varp[:], in1=y2[:],
                                            op=OP.mult)
                    nc.vector.tensor_scalar(
                        out=vy2[:], in0=vy2[:], scalar1=-0.5, scalar2=1.5,
                        op0=OP.mult, op1=OP.add)
                    y1 = sttp.tile([128, 1], F32, tag="y1")
                    nc.vector.tensor_tensor(out=y1[:], in0=y0[:], in1=vy2[:],
                                            op=OP.mult)
                    nc.vector.tensor_tensor(out=abuf[:, ct:ct + 1],
                                            in0=bng_t[:, ct:ct + 1],
                                            in1=y1[:], op=OP.mult)
                    mua = sttp.tile([128, 1], F32, tag="mua")
                    nc.vector.tensor_tensor(out=mua[:], in0=mu[:],
                                            in1=abuf[:, ct:ct + 1],
                                            op=OP.mult)
                    nc.vector.tensor_tensor(out=bbuf[:, ct:ct + 1],
                                            in0=bnb_t[:, ct:ct + 1],
                                            in1=mua[:], op=OP.subtract)
            for ct in range(cfg.CHT):
                nc.vector.tensor_scalar(
                    out=nrm[:, ct, :], in0=hid[:, ct, :],
                    scalar1=abuf[:, ct:ct + 1], scalar2=bbuf[:, ct:ct + 1],
                    op0=OP.mult, op1=OP.add)

        # ---- phase D: MLP + masked max-pool + linear -------------------
        with tc.tile_pool(name="mwtp", bufs=1) as mwtp, \
             tc.tile_pool(name="tailc", bufs=1) as tailc, \
             tc.tile_pool(name="qp", bufs=3) as qp, \
             tc.tile_pool(name="pmlp", bufs=4, space="PSUM") as pmlpp, \
             tc.tile_pool(name="pfin", bufs=1, space="PSUM") as pfinp:
            mwt_t = [mwtp.tile([128, 2 * H], BF16, name=f"mwt{kt}")
                     for kt in range(cfg.CHT)]
            for kt in range(cfg.CHT):
                nc.sync.dma_start(mwt_t[kt][:], mwt_d[kt * 128:(kt + 1) * 128, :])
            mb65_t = tailc.tile([128, cfg.CHT], F32)
            nc.sync.dma_start(mb65_t[:], mb65_d[:])
            lwt_t = [tailc.tile([128, C], F32, name=f"lwt{kt}")
                     for kt in range(cfg.CHT)]
            for kt in range(cfg.CHT):
                nc.sync.dma_start(lwt_t[kt][:], lwt_d[kt * 128:(kt + 1) * 128, :])
            lb_t = tailc.tile([128, C], F32)
            nc.sync.dma_start(lb_t[:BC, :], lb4_d[:, :])
            pld = [tailc.tile([128, BC], F32, name=f"pld{mt}")
                   for mt in range(cfg.CHT)]
            for c in range(BC):
                hc = slice(c * S, (c + 1) * S)
                for mt in range(cfg.CHT):
                    pm = pmlpp.tile([128, S], F32, space="PSUM", tag="pm")
                    for kt in range(cfg.CHT):
                        nc.tensor.matmul(
                            pm[:],
                            lhsT=mwt_t[kt][:, mt * 128:(mt + 1) * 128],
                            rhs=nrm[:, kt, hc],
                            start=(kt == 0), stop=(kt == cfg.CHT - 1))
                    # q = (pm + mlp_b) + moff  (mask-mult dropped: dead
                    # positions get -65500 +- |mlp_h|, max unaffected)
                    q = qp.tile([128, S], F32, tag="q")
                    nc.vector.scalar_tensor_tensor(
                        out=q[:], in0=pm[:], scalar=mb65_t[:, mt:mt + 1],
                        in1=moff_t[:, hc], op0=OP.add, op1=OP.add)
                    nc.vector.tensor_reduce(
                        out=pld[mt][:, c:c + 1], in_=q[:], axis=AX.X,
                        op=OP.max)
            pf = pfinp.tile([128, C], F32, space="PSUM")
            for mt in range(cfg.CHT):
                nc.tensor.matmul(pf[:BC, :], lhsT=pld[mt][:, :BC],
                                 rhs=lwt_t[mt][:, :],
                                 start=(mt == 0), stop=(mt == cfg.CHT - 1))
            ob = tailc.tile([128, C], F32)
            nc.vector.tensor_tensor(out=ob[:BC, :], in0=pf[:BC, :],
                                    in1=lb_t[:BC, :], op=OP.add)
            nc.sync.dma_start(out_d[:, :], ob[:BC, :])
        nrmp.release()
        rep_ctx.__exit__(None, None, None)
        if cfg.repeat > 1:
            if cfg.use_fp8:
                h8p.release()
            hp.release()
            xgp.release()
        maskp.release()
        hidp.release()
        constp.release()

    nc.compile()
    return nc


def prep_inputs(inputs, cfg: Cfg):
    """Host-side sharding/prep. Returns in_maps (one dict per core)."""
    B, S, W, E, H, C = cfg.B, cfg.S, cfg.W, cfg.E, cfg.H, cfg.C
    x = np.asarray(inputs["x"]).astype(np.int64)
    emb = np.asarray(inputs["emb"], dtype=np.float32)
    mask = (x > 0).astype(np.float32)                       # [B, S]

    def bf(a):
        return np.ascontiguousarray(np.asarray(a, np.float32)
                                    .astype(np.float16))

    def f32(a):
        return np.ascontiguousarray(np.asarray(a, dtype=np.float32))

    shared = {}
    for d, sfx in enumerate("fb"):
        W_ih = np.asarray(inputs[f"W_ih_{sfx}"], np.float32)
        W_hh = np.asarray(inputs[f"W_hh_{sfx}"], np.float32)
        b_ih = np.asarray(inputs[f"b_ih_{sfx}"], np.float32)
        b_hh = np.asarray(inputs[f"b_hh_{sfx}"], np.float32)
        shared[f"wih{d}"] = bf(W_ih.T)                       # [E, G]
        # [128, 2, G] with [p, j, g] = W_hh.T[j*128+p, g]
        shared[f"whh2{d}"] = bf(W_hh.T.reshape(2, 128, cfg.G)
                                .transpose(1, 0, 2).reshape(128, 2 * cfg.G))
        bfold = b_ih.copy()
        bfold[:2 * H] += b_hh[:2 * H]                        # r,z gates
        shared[f"bg{d}"] = f32(bfold.reshape(cfg.GS, 128).T)  # [128, GS]
        shared[f"bhn{d}"] = f32(b_hh[2 * H:].reshape(cfg.HK, 128).T)
    shared["identf"] = f32(np.eye(128))
    shared["identb"] = bf(np.eye(128))
    shared["bng"] = f32(np.asarray(inputs["bn_gamma"], np.float32)
                        .reshape(cfg.CHT, 128).T)
    shared["bnb"] = f32(np.asarray(inputs["bn_beta"], np.float32)
                        .reshape(cfg.CHT, 128).T)
    mlp_b = np.asarray(inputs["mlp_b"], np.float32)
    shared["mb65"] = f32(mlp_b.reshape(cfg.CHT, 128).T)
    shared["mwt"] = bf(np.asarray(inputs["mlp_W"], np.float32).T)
    lin_W = np.asarray(inputs["lin_W"], np.float32)
    lin_b = np.asarray(inputs["lin_b"], np.float32)
    shared["lwt"] = f32(lin_W.T)                             # [2H, C]
    shared["lb4"] = f32(np.broadcast_to(lin_b[None, :], (cfg.BC, C)))

    in_maps = []
    for core in range(cfg.n_cores):
        rows = x[core * cfg.BC:(core + 1) * cfg.BC]          # [BC, S]
        ids = np.zeros((cfg.BC, cfg.SEG), np.int64)
        ids[:, W - 1:W - 1 + S] = rows
        ids = ids.reshape(-1)                                # [TC]
        uids, inv = np.unique(ids, return_inverse=True)
        pt = np.zeros((cfg.TC, E), np.float16)
        pt[:len(uids)] = emb[uids].astype(np.float16)
        m = {k: v for k, v in shared.items()}
        m["ptab"] = pt
        m["ids"] = np.ascontiguousarray(inv.astype(np.int32)[:, None])
        mrow = mask[core * cfg.BC:(core + 1) * cfg.BC].reshape(-1)  # [NR]
        m["maskin"] = np.ascontiguousarray(
            np.broadcast_to(mrow[None, :], (128, cfg.NR)).astype(np.float16))
        in_maps.append(m)
    return in_maps


_CACHE = {}


def get_compiled(cfg: Cfg | None = None):
    key = "default" if cfg is None else id(cfg)
    if key not in _CACHE:
        _CACHE[key] = build(cfg or Cfg())
    return _CACHE[key]


def kernel(**inputs) -> np.ndarray:
    cfg = Cfg()
    nc = get_compiled(None)
    in_maps = prep_inputs(inputs, cfg)
    res = run_bass_kernel_spmd(nc, in_maps, core_ids=list(range(cfg.n_cores)))
    return np.concatenate([res.results[i]["out"] for i in range(cfg.n_cores)],
                          axis=0).astype(np.float32)


# revision 18
# speedup vs baseline: 1.6602x; 1.6602x over previous
"""Trainium2 Bass kernel for nn_DRNN_75204877353433 (v2).

Windowed bidirectional GRU (W=15) over [B=32, S=512] token ids ->
batch-norm (training stats over B,S) -> MLP -> masked max-pool -> linear.

v2 strategy (8 NeuronCores, data-parallel over batch; BC=4 rows/core):
  - ACT (scalar) engine is the steady-state bottleneck (3 sigmoid/tanh
    elements per gate-dim per step, 1 elem/cycle/lane, no fast mode), so
    the scan is restructured so every other engine stays under ACT's
    ~3.2us/iteration period:
      * recurrent matmuls h @ W_hh run in fp8e4 with DoubleRow perf mode
        (contraction 256 in one pass, 0.5 cycles/row): h keeps an fp16
        master copy (accuracy) and the Pool engine writes an fp8 mirror
        used only as matmul input;
      * xr/xz injections into PSUM stay on the PE as identity matmuls
        (cheapest engine for a +tile into PSUM);
      * sigmoids are merged per k-tile ([r|z] in one 2-bank PSUM tile),
        tanh merged across k;
      * elementwise chain is split DVE/Pool so neither exceeds ACT.
  - program order is stage-interleaved across the 8 (c,d) streams per
    window step so every engine sees a homogeneous run of ready work
    instead of serialized per-stream chains (the v1 bottleneck).
"""

import sys

for _p in ("/opt/trn_rl_repo",):
    if _p not in sys.path:
        sys.path.insert(0, _p)

import numpy as np

from concourse import bacc, mybir, tile
from concourse.bass import IndirectOffsetOnAxis
from concourse.bass_utils import run_bass_kernel_spmd

F32 = mybir.dt.float32
BF16 = mybir.dt.float16    # 16-bit compute dtype: fp16 (10-bit mantissa)
FP8 = mybir.dt.float8e4
I32 = mybir.dt.int32
AF = mybir.ActivationFunctionType
OP = mybir.AluOpType
AX = mybir.AxisListType
DR = mybir.MatmulPerfMode.DoubleRow


class Cfg:
    def __init__(self, B=32, S=512, W=15, E=300, H=256, C=2, n_cores=8,
                 use_cc=True, use_fp8=True, repeat=1,
                 prz_bufs=2, pn_bufs=2, gate_bufs=4, group=3,
                 pool_full=True):
        self.B, self.S, self.W, self.E, self.H, self.C = B, S, W, E, H, C
        self.n_cores = n_cores
        self.use_cc = use_cc
        self.use_fp8 = use_fp8
        self.repeat = repeat
        self.prz_bufs = prz_bufs
        self.pn_bufs = pn_bufs
        self.gate_bufs = gate_bufs
        self.group = group
        self.pool_full = pool_full
        self.G = 3 * H
        self.BC = B // n_cores                      # batch rows per core
        seg = S + 2 * (W - 1)                       # valid token cols per row
        self.SEG = seg
        while (self.BC * self.SEG) % 128:
            self.SEG += 1
        self.TC = self.BC * self.SEG                # token cols per core
        self.NT = self.TC // 128                    # gather tiles
        self.NR = self.BC * S                       # window rows per core
        self.HK = (H + 127) // 128                  # H partition tiles (2)
        self.GS = self.G // 128                     # G subtiles (6)
        self.EK = [(k * 128, min(128, E - k * 128))
                   for k in range((E + 127) // 128)]
        self.CHT = (2 * H) // 128                   # hidden channel tiles (4)
        self.XCH = [(i * 512, min(512, self.TC - i * 512))
                    for i in range((self.TC + 511) // 512)]
        assert H % 128 == 0 and self.G % 128 == 0 and self.HK == 2


def build(cfg: Cfg):
    """Build + bacc-compile the Bass program. Returns nc."""
    nc = bacc.Bacc("TRN2", target_bir_lowering=False, debug=False,
                   enable_asserts=False, num_devices=cfg.n_cores)
    _eps_t = nc.alloc_sbuf_tensor("const-eps", [128, 1], F32)
    nc.gpsimd.memset(_eps_t.ap(), 1e-5)
    nc.const_aps.aps[(F32, 1e-5)] = _eps_t.ap()
    nc.all_engine_barrier()
    B, S, W, E, H, C = cfg.B, cfg.S, cfg.W, cfg.E, cfg.H, cfg.C
    BC, SEG, TC, NT, NR, HK, GS = (cfg.BC, cfg.SEG, cfg.TC, cfg.NT, cfg.NR,
                                   cfg.HK, cfg.GS)

    def din(name, shape, dt):
        return nc.dram_tensor(name, shape, dt, kind="ExternalInput").ap()

    ptab = din("ptab", [TC, E], BF16)
    ids = din("ids", [TC, 1], I32)
    maskin = din("maskin", [128, NR], BF16)
    wih = [din(f"wih{d}", [E, cfg.G], BF16) for d in range(2)]
    whh2 = [din(f"whh2{d}", [128, 2 * cfg.G], BF16) for d in range(2)]
    bgd = [din(f"bg{d}", [128, GS], F32) for d in range(2)]
    bhnd = [din(f"bhn{d}", [128, HK], F32) for d in range(2)]
    identf_d = din("identf", [128, 128], F32)
    identb_d = din("identb", [128, 128], BF16)
    bng_d = din("bng", [128, cfg.CHT], F32)
    bnb_d = din("bnb", [128, cfg.CHT], F32)
    mb65_d = din("mb65", [128, cfg.CHT], F32)
    mwt_d = din("mwt", [2 * H, 2 * H], BF16)
    lwt_d = din("lwt", [2 * H, C], F32)
    lb4_d = din("lb4", [BC, C], F32)
    out_d = nc.dram_tensor("out", [BC, C], F32, kind="ExternalOutput").ap()

    inv_n = 1.0 / float(B * S)

    with tile.TileContext(nc) as tc:
        # ---- persistent constants -------------------------------------
        constp = tc.alloc_tile_pool(name="const", bufs=1)
        identf = constp.tile([128, 128], F32)
        identb = constp.tile([128, 128], BF16)
        nc.sync.dma_start(identf[:], identf_d[:])
        nc.sync.dma_start(identb[:], identb_d[:])
        whh16 = [constp.tile([128, 2, cfg.G], BF16, name=f"whh16_{d}")
                 for d in range(2)]
        for d in range(2):
            nc.sync.dma_start(whh16[d][:], whh2[d][:].rearrange(
                "p (j g) -> p j g", j=2))
        if cfg.use_fp8:
            whh8 = [constp.tile([128, 2, cfg.G], FP8, name=f"whh8_{d}")
                    for d in range(2)]
            for d in range(2):
                nc.vector.tensor_copy(out=whh8[d][:], in_=whh16[d][:])
        bg_t = [constp.tile([128, GS], F32, name=f"bg{d}") for d in range(2)]
        bhn_t = [constp.tile([128, HK], F32, name=f"bhn{d}") for d in range(2)]
        for d in range(2):
            nc.sync.dma_start(bg_t[d][:], bgd[d][:])
            nc.sync.dma_start(bhn_t[d][:], bhnd[d][:])

        # persistent state tiles
        xgp = tc.alloc_tile_pool(name="xg", bufs=1)
        xg = [xgp.tile([128, GS, TC], BF16, name=f"xg{d}") for d in range(2)]
        hp = tc.alloc_tile_pool(name="h", bufs=1)
        h_m = [hp.tile([128, 2, NR], BF16, name=f"hm{d}") for d in range(2)]
        if cfg.use_fp8:
            h8p = tc.alloc_tile_pool(name="h8", bufs=1)
            h_8 = [h8p.tile([128, 2, NR], FP8, name=f"h8{d}") for d in range(2)]
        hidp = tc.alloc_tile_pool(name="hid", bufs=1, side="right")
        hid = hidp.tile([128, cfg.CHT, NR], BF16)
        maskp = tc.alloc_tile_pool(name="maskp", bufs=1, side="right")
        mask_t = maskp.tile([128, NR], BF16)
        moff_t = maskp.tile([128, NR], F32)

        from contextlib import nullcontext
        rep_ctx = tc.For_i(0, cfg.repeat, 1) if cfg.repeat > 1 \
            else nullcontext()
        rep_ctx.__enter__()

        nc.sync.dma_start(mask_t[:], maskin[:])
        # moff = (mask-1)*65500 : 0 live, -65500 dead (phase D additive)
        nc.vector.tensor_scalar(
            out=moff_t[:], in0=mask_t[:], scalar1=1.0, scalar2=65500.0,
            op0=OP.subtract, op1=OP.mult)

        # ---- phase A: gather + transpose + xg precompute ---------------
        with tc.tile_pool(name="wihp", bufs=1) as wihp, \
             tc.tile_pool(name="idsp", bufs=2) as idsp, \
             tc.tile_pool(name="eraw", bufs=3) as erawp, \
             tc.tile_pool(name="eT", bufs=1) as eTp, \
             tc.tile_pool(name="tpsum", bufs=2, space="PSUM") as tpsump, \
             tc.tile_pool(name="xgpsum", bufs=4, space="PSUM") as xgpsump:
            wih_t = [[wihp.tile([128, cfg.G], BF16, name=f"wih{d}_{k}")
                      for k in range(len(cfg.EK))] for d in range(2)]
            for d in range(2):
                for k, (e0, ew) in enumerate(cfg.EK):
                    nc.sync.dma_start(wih_t[d][k][:ew, :], wih[d][e0:e0 + ew, :])
            eT = eTp.tile([128, len(cfg.EK), TC], BF16)
            for t in range(NT):
                idt = idsp.tile([128, 1], I32)
                nc.sync.dma_start(idt[:], ids[t * 128:(t + 1) * 128, :])
                er = erawp.tile([128, E], BF16)
                nc.gpsimd.indirect_dma_start(
                    out=er[:], out_offset=None, in_=ptab[:],
                    in_offset=IndirectOffsetOnAxis(ap=idt[:, :1], axis=0),
                )
                tp = tpsump.tile([128, len(cfg.EK), 128], BF16, space="PSUM")
                for k, (e0, ew) in enumerate(cfg.EK):
                    nc.tensor.transpose(out=tp[:ew, k, :],
                                        in_=er[:, e0:e0 + ew],
                                        identity=identb[:])
                nc.vector.tensor_copy(
                    out=eT[:, :, t * 128:(t + 1) * 128], in_=tp[:])
            nkk = len(cfg.EK)
            cnt = 0
            for d in range(2):
                for g in range(GS):
                    for (c0, cw) in cfg.XCH:
                        p = xgpsump.tile([128, 512], F32, space="PSUM")
                        for k, (e0, ew) in enumerate(cfg.EK):
                            nc.tensor.matmul(
                                p[:, :cw],
                                lhsT=wih_t[d][k][:ew, g * 128:(g + 1) * 128],
                                rhs=eT[:ew, k, c0:c0 + cw],
                                start=(k == 0), stop=(k == nkk - 1))
                        dst = xg[d][:, g, c0:c0 + cw]
                        if cnt % 3 == 2:
                            nc.vector.tensor_single_scalar(
                                out=dst, in_=p[:, :cw],
                                scalar=bg_t[d][:, g:g + 1], op=OP.add)
                        else:
                            nc.scalar.activation(
                                out=dst, in_=p[:, :cw],
                                func=AF.Identity, bias=bg_t[d][:, g:g + 1])
                        cnt += 1

        # ---- phase B: the windowed GRU scan ----------------------------
        # streams: (c, d); g-tile order in xg/whh: [r0, r1, z0, z1, n0, n1]
        streams = [(c, d) for c in range(BC) for d in range(2)]

        def base_of(c, d, w):
            off = w if d == 0 else 2 * (W - 1) - w
            return c * SEG + off

        GB = cfg.gate_bufs
        with tc.tile_pool(name="prz", bufs=cfg.prz_bufs, space="PSUM") as przp, \
             tc.tile_pool(name="pn", bufs=cfg.pn_bufs, space="PSUM") as pnp, \
             tc.tile_pool(name="rz4", bufs=GB) as rzp, \
             tc.tile_pool(name="tsb", bufs=GB) as tsp, \
             tc.tile_pool(name="tnb", bufs=GB) as tnp, \
             tc.tile_pool(name="nb", bufs=GB) as nbp, \
             tc.tile_pool(name="db", bufs=GB) as dbp, \
             tc.tile_pool(name="eb", bufs=GB) as ebp:

            groups = [streams[i:i + cfg.group]
                      for i in range(0, len(streams), cfg.group)]

            def emit_w0(grp):
                """h1 = (1-z)*n, n = tanh(xn + r*bhn); h0 = 0."""
                rzs, ts, tns, ns = {}, {}, {}, {}
                for s in grp:
                    c, d = s
                    b0 = base_of(c, d, 0)
                    rz4 = rzp.tile([128, 4, S], BF16, tag="rz")
                    # blocks [rk0, zk0, rk1, zk1]; xg g-order [r0,r1,z0,z1]
                    for k in range(2):
                        nc.scalar.activation(
                            out=rz4[:, 2 * k, :], in_=xg[d][:, k, b0:b0 + S],
                            func=AF.Sigmoid)
                        nc.scalar.activation(
                            out=rz4[:, 2 * k + 1, :],
                            in_=xg[d][:, 2 + k, b0:b0 + S], func=AF.Sigmoid)
                    rzs[s] = rz4
                for s in grp:
                    c, d = s
                    t_ = tsp.tile([128, 2, S], BF16, tag="t")
                    for k in range(2):
                        nc.vector.tensor_single_scalar(
                            out=t_[:, k, :], in_=rzs[s][:, 2 * k, :],
                            scalar=bhn_t[d][:, k:k + 1], op=OP.mult)
                    ts[s] = t_
                for s in grp:
                    c, d = s
                    b0 = base_of(c, d, 0)
                    tn_ = tnp.tile([128, 2, S], BF16, tag="tn")
                    eng = nc.gpsimd if cfg.pool_full else nc.vector
                    eng.tensor_tensor(
                        out=tn_[:], in0=ts[s][:], in1=xg[d][:, 4:6, b0:b0 + S],
                        op=OP.add)
                    tns[s] = tn_
                for s in grp:
                    n_ = nbp.tile([128, 2, S], BF16, tag="n")
                    nc.scalar.activation(out=n_[:], in_=tns[s][:], func=AF.Tanh)
                    ns[s] = n_
                for s in grp:
                    c, d = s
                    hc = slice(c * S, (c + 1) * S)
                    nc.vector.scalar_tensor_tensor(
                        out=h_m[d][:, :, hc], in0=rzs[s][:, 1::2, :],
                        scalar=1.0, in1=ns[s][:],
                        op0=OP.subtract, op1=OP.mult)
                if cfg.use_fp8:
                    for s in grp:
                        c, d = s
                        hc = slice(c * S, (c + 1) * S)
                        eng = nc.gpsimd if cfg.pool_full else nc.vector
                        eng.tensor_copy(out=h_8[d][:, :, hc],
                                        in_=h_m[d][:, :, hc])

            def emit_w(w, grp):
                last = (w == W - 1)
                przs, pns, rzs, ts, tns, ns, ds, es = ({}, {}, {}, {}, {},
                                                       {}, {}, {})
                for s in grp:
                    c, d = s
                    b0 = base_of(c, d, w)
                    hc = slice(c * S, (c + 1) * S)
                    prz = [przp.tile([128, 2, S], F32, space="PSUM",
                                     tag="prz", name=f"prz{k}")
                           for k in range(2)]
                    pn = pnp.tile([128, 2, S], F32, space="PSUM", tag="pn")
                    # xr/xz injections (identity matmuls, start=True)
                    for k in range(2):
                        nc.tensor.matmul(prz[k][:, 0, :], lhsT=identb[:],
                                         rhs=xg[d][:, k, b0:b0 + S],
                                         start=True, stop=False)
                        nc.tensor.matmul(prz[k][:, 1, :], lhsT=identb[:],
                                         rhs=xg[d][:, 2 + k, b0:b0 + S],
                                         start=True, stop=False)
                    # recurrent matmuls
                    if cfg.use_fp8:
                        rhs8 = h_8[d][:, :, hc]
                        for k in range(2):
                            nc.tensor.matmul(
                                prz[k][:, 0, :],
                                lhsT=whh8[d][:, :, k * 128:(k + 1) * 128],
                                rhs=rhs8, start=False, stop=True, perf_mode=DR)
                            nc.tensor.matmul(
                                prz[k][:, 1, :],
                                lhsT=whh8[d][:, :, 256 + k * 128:256 + (k + 1) * 128],
                                rhs=rhs8, start=False, stop=True, perf_mode=DR)
                            nc.tensor.matmul(
                                pn[:, k, :],
                                lhsT=whh8[d][:, :, 512 + k * 128:512 + (k + 1) * 128],
                                rhs=rhs8, start=True, stop=True, perf_mode=DR)
                    else:
                        for k in range(2):
                            for kk in range(2):
                                lw = whh16[d][:, kk, :]
                                rh = h_m[d][:, kk, hc]
                                nc.tensor.matmul(
                                    prz[k][:, 0, :],
                                    lhsT=lw[:, k * 128:(k + 1) * 128],
                                    rhs=rh, start=False, stop=(kk == 1))
                                nc.tensor.matmul(
                                    prz[k][:, 1, :],
                                    lhsT=lw[:, 256 + k * 128:256 + (k + 1) * 128],
                                    rhs=rh, start=False, stop=(kk == 1))
                                nc.tensor.matmul(
                                    pn[:, k, :],
                                    lhsT=lw[:, 512 + k * 128:512 + (k + 1) * 128],
                                    rhs=rh, start=(kk == 0), stop=(kk == 1))
                    przs[s], pns[s] = prz, pn
                for s in grp:
                    rz4 = rzp.tile([128, 4, S], BF16, tag="rz")
                    for k in range(2):
                        nc.scalar.activation(
                            out=rz4[:, 2 * k:2 * k + 2, :],
                            in_=przs[s][k][:], func=AF.Sigmoid)
                    rzs[s] = rz4
                for s in grp:
                    c, d = s
                    t_ = tsp.tile([128, 2, S], BF16, tag="t")
                    for k in range(2):
                        nc.vector.scalar_tensor_tensor(
                            out=t_[:, k, :], in0=pns[s][:, k, :],
                            scalar=bhn_t[d][:, k:k + 1],
                            in1=rzs[s][:, 2 * k, :], op0=OP.add, op1=OP.mult)
                    ts[s] = t_
                for s in grp:
                    c, d = s
                    b0 = base_of(c, d, w)
                    tn_ = tnp.tile([128, 2, S], BF16, tag="tn")
                    eng = nc.gpsimd if cfg.pool_full else nc.vector
                    eng.tensor_tensor(
                        out=tn_[:], in0=ts[s][:],
                        in1=xg[d][:, 4:6, b0:b0 + S], op=OP.add)
                    tns[s] = tn_
                for s in grp:
                    n_ = nbp.tile([128, 2, S], BF16, tag="n")
                    nc.scalar.activation(out=n_[:], in_=tns[s][:], func=AF.Tanh)
                    ns[s] = n_
                for s in grp:
                    c, d = s
                    hc = slice(c * S, (c + 1) * S)
                    d_ = dbp.tile([128, 2, S], BF16, tag="d")
                    nc.gpsimd.tensor_tensor(
                        out=d_[:], in0=h_m[d][:, :, hc], in1=ns[s][:],
                        op=OP.subtract)
                    ds[s] = d_
                for s in grp:
                    e_ = ebp.tile([128, 2, S], BF16, tag="e")
                    nc.vector.tensor_tensor(
                        out=e_[:], in0=rzs[s][:, 1::2, :], in1=ds[s][:],
                        op=OP.mult)
                    es[s] = e_
                for s in grp:
                    c, d = s
                    hc = slice(c * S, (c + 1) * S)
                    dest = hid[:, 2 * d:2 * d + 2, hc] if last \
                        else h_m[d][:, :, hc]
                    nc.vector.tensor_tensor(
                        out=dest, in0=ns[s][:], in1=es[s][:], op=OP.add)
                if cfg.use_fp8 and not last:
                    for s in grp:
                        c, d = s
                        hc = slice(c * S, (c + 1) * S)
                        eng = nc.gpsimd if cfg.pool_full else nc.vector
                        eng.tensor_copy(out=h_8[d][:, :, hc],
                                        in_=h_m[d][:, :, hc])

            for grp in groups:
                emit_w0(grp)
            for w in range(1, W):
                for grp in groups:
                    emit_w(w, grp)

        if cfg.repeat == 1:
            if cfg.use_fp8:
                h8p.release()
            hp.release()
            xgp.release()

        # ---- phase C: BN stats + AllReduce + affine --------------------
        nrmp = tc.alloc_tile_pool(name="nrm", bufs=1, side="right")
        nrm = nrmp.tile([128, cfg.CHT, NR], BF16)
        with tc.tile_pool(name="scr", bufs=2) as scrp, \
             tc.tile_pool(name="stat", bufs=1) as statp, \
             tc.tile_pool(name="dram", bufs=1, space="DRAM") as dramp:
            sums = statp.tile([128, 2 * cfg.CHT], F32)
            dummy = statp.tile([128, 1], F32)
            for ct in range(cfg.CHT):
                sc = scrp.tile([128, NR], BF16, tag="scr")
                nc.vector.tensor_tensor(out=sc[:], in0=hid[:, ct, :],
                                        in1=mask_t[:], op=OP.mult)
                nc.vector.tensor_reduce(out=sums[:, ct:ct + 1],
                                        in_=sc[:], axis=AX.X, op=OP.add)
                sq = scrp.tile([128, NR], BF16, tag="scr2")
                nc.vector.tensor_tensor(out=sq[:], in0=sc[:], in1=sc[:],
                                        op=OP.mult)
                nc.vector.tensor_reduce(
                    out=sums[:, cfg.CHT + ct:cfg.CHT + ct + 1],
                    in_=sq[:], axis=AX.X, op=OP.add)
            gsums = statp.tile([128, 2 * cfg.CHT], F32)
            if cfg.use_cc:
                bnc_in = dramp.tile([128, 2 * cfg.CHT], F32)
                bnc_out = dramp.tile([128, 2 * cfg.CHT], F32,
                                     addr_space="Shared")
                nc.gpsimd.dma_start(bnc_in[:], sums[:])
                nc.gpsimd.collective_compute(
                    "AllReduce", OP.add,
                    replica_groups=[list(range(cfg.n_cores))],
                    ins=[bnc_in.opt()], outs=[bnc_out.opt()])
                nc.gpsimd.dma_start(gsums[:], bnc_out[:])
            else:
                nc.vector.tensor_copy(out=gsums[:], in_=sums[:])

            bng_t = statp.tile([128, cfg.CHT], F32)
            bnb_t = statp.tile([128, cfg.CHT], F32)
            nc.sync.dma_start(bng_t[:], bng_d[:])
            nc.sync.dma_start(bnb_t[:], bnb_d[:])
            abuf = statp.tile([128, cfg.CHT], F32)
            bbuf = statp.tile([128, cfg.CHT], F32)
            with nc.allow_low_precision("bn 1/sqrt + NR refine"), \
                 tc.tile_pool(name="stt", bufs=2) as sttp:
                for ct in range(cfg.CHT):
                    gs_s = gsums[:, ct:ct + 1]
                    gs_q = gsums[:, cfg.CHT + ct:cfg.CHT + ct + 1]
                    mu = sttp.tile([128, 1], F32, tag="mu")
                    nc.scalar.mul(mu[:], gs_s, inv_n)
                    mq = sttp.tile([128, 1], F32, tag="mq")
                    nc.scalar.square(mq[:], mu[:])
                    varp = sttp.tile([128, 1], F32, tag="var")
                    nc.vector.scalar_tensor_tensor(
                        out=varp[:], in0=gs_q, scalar=inv_n, in1=mq[:],
                        op0=OP.mult, op1=OP.subtract)
                    nc.scalar.add(varp[:], varp[:], 1e-5)
                    sd = sttp.tile([128, 1], F32, tag="sd")
                    nc.scalar.sqrt(sd[:], varp[:])
                    y0 = sttp.tile([128, 1], F32, tag="y0")
                    nc.vector.reciprocal(y0[:], sd[:])
                    y2 = sttp.tile([128, 1], F32, tag="y2")
                    nc.vector.tensor_tensor(out=y2[:], in0=y0[:], in1=y0[:],
                                            op=OP.mult)
                    vy2 = sttp.tile([128, 1], F32, tag="vy2")
                    nc.vector.tensor_tensor(out=vy2[:], in0=varp[:], in1=y2[:],
                                            op=OP.mult)
                    nc.vector.tensor_scalar(
                        out=vy2[:], in0=vy2[:], scalar1=-0.5, scalar2=1.5,
                        op0=OP.mult, op1=OP.add)
                    y1 = sttp.tile([128, 1], F32, tag="y1")
                    nc.vector.tensor_tensor(out=y1[:], in0=y0[:], in1=vy2[:],
                                            op=OP.mult)
                    nc.vector.tensor_tensor(out=abuf[:, ct:ct + 1],
                                            in0=bng_t[:, ct:ct + 1],
                                            in1=y1[:], op=OP.mult)
                    mua = sttp.tile([128, 1], F32, tag="mua")
                    nc.vector.tensor_tensor(out=mua[:], in0=mu[:],
                                            in1=abuf[:, ct:ct + 1],
                                            op=OP.mult)
                    nc.vector.tensor_tensor(out=bbuf[:, ct:ct + 1],
                                            in0=bnb_t[:, ct:ct + 1],
                                            in1=mua[:], op=OP.subtract)
            for ct in range(cfg.CHT):
                nc.vector.tensor_scalar(
                    out=nrm[:, ct, :], in0=hid[:, ct, :],
                    scalar1=abuf[:, ct:ct + 1], scalar2=bbuf[:, ct:ct + 1],
                    op0=OP.mult, op1=OP.add)

        # ---- phase D: MLP + masked max-pool + linear -------------------
        with tc.tile_pool(name="mwtp", bufs=1) as mwtp, \
             tc.tile_pool(name="tailc", bufs=1) as tailc, \
             tc.tile_pool(name="qp", bufs=3) as qp, \
             tc.tile_pool(name="pmlp", bufs=4, space="PSUM") as pmlpp, \
             tc.tile_pool(name="pfin", bufs=1, space="PSUM") as pfinp:
            mwt_t = [mwtp.tile([128, 2 * H], BF16, name=f"mwt{kt}")
                     for kt in range(cfg.CHT)]
            for kt in range(cfg.CHT):
                nc.sync.dma_start(mwt_t[kt][:], mwt_d[kt * 128:(kt + 1) * 128, :])
            mb65_t = tailc.tile([128, cfg.CHT], F32)
            nc.sync.dma_start(mb65_t[:], mb65_d[:])
            lwt_t = [tailc.tile([128, C], F32, name=f"lwt{kt}")
                     for kt in range(cfg.CHT)]
            for kt in range(cfg.CHT):
                nc.sync.dma_start(lwt_t[kt][:], lwt_d[kt * 128:(kt + 1) * 128, :])
            lb_t = tailc.tile([128, C], F32)
            nc.sync.dma_start(lb_t[:BC, :], lb4_d[:, :])
            pld = [tailc.tile([128, BC], F32, name=f"pld{mt}")
                   for mt in range(cfg.CHT)]
            for c in range(BC):
                hc = slice(c * S, (c + 1) * S)
                for mt in range(cfg.CHT):
                    pm = pmlpp.tile([128, S], F32, space="PSUM", tag="pm")
                    for kt in range(cfg.CHT):
                        nc.tensor.matmul(
                            pm[:],
                            lhsT=mwt_t[kt][:, mt * 128:(mt + 1) * 128],
                            rhs=nrm[:, kt, hc],
                            start=(kt == 0), stop=(kt == cfg.CHT - 1))
                    # q = (pm + mlp_b) + moff  (mask-mult dropped: dead
                    # positions get -65500 +- |mlp_h|, max unaffected)
                    q = qp.tile([128, S], F32, tag="q")
                    nc.vector.scalar_tensor_tensor(
                        out=q[:], in0=pm[:], scalar=mb65_t[:, mt:mt + 1],
                        in1=moff_t[:, hc], op0=OP.add, op1=OP.add)
                    nc.vector.tensor_reduce(
                        out=pld[mt][:, c:c + 1], in_=q[:], axis=AX.X,
                        op=OP.max)
            pf = pfinp.tile([128, C], F32, space="PSUM")
            for mt in range(cfg.CHT):
                nc.tensor.matmul(pf[:BC, :], lhsT=pld[mt][:, :BC],
                                 rhs=lwt_t[mt][:, :],
                                 start=(mt == 0), stop=(mt == cfg.CHT - 1))
            ob = tailc.tile([128, C], F32)
            nc.vector.tensor_tensor(out=ob[:BC, :], in0=pf[:BC, :],
                                    in1=lb_t[:BC, :], op=OP.add)
            nc.sync.dma_start(out_d[:, :], ob[:BC, :])
        nrmp.release()
        rep_ctx.__exit__(None, None, None)
        if cfg.repeat > 1:
            if cfg.use_fp8:
                h8p.release()
            hp.release()
            xgp.release()
        maskp.release()
        hidp.release()
        constp.release()

    nc.compile()
    return nc


def prep_inputs(inputs, cfg: Cfg):
    """Host-side sharding/prep. Returns in_maps (one dict per core)."""
    B, S, W, E, H, C = cfg.B, cfg.S, cfg.W, cfg.E, cfg.H, cfg.C
    x = np.asarray(inputs["x"]).astype(np.int64)
    emb = np.asarray(inputs["emb"], dtype=np.float32)
    mask = (x > 0).astype(np.float32)                       # [B, S]

    def bf(a):
        return np.ascontiguousarray(np.asarray(a, np.float32)
                                    .astype(np.float16))

    def f32(a):
        return np.ascontiguousarray(np.asarray(a, dtype=np.float32))

    shared = {}
    for d, sfx in enumerate("fb"):
        W_ih = np.asarray(inputs[f"W_ih_{sfx}"], np.float32)
        W_hh = np.asarray(inputs[f"W_hh_{sfx}"], np.float32)
        b_ih = np.asarray(inputs[f"b_ih_{sfx}"], np.float32)
        b_hh = np.asarray(inputs[f"b_hh_{sfx}"], np.float32)
        shared[f"wih{d}"] = bf(W_ih.T)                       # [E, G]
        # [128, 2, G] with [p, j, g] = W_hh.T[j*128+p, g]
        shared[f"whh2{d}"] = bf(W_hh.T.reshape(2, 128, cfg.G)
                                .transpose(1, 0, 2).reshape(128, 2 * cfg.G))
        bfold = b_ih.copy()
        bfold[:2 * H] += b_hh[:2 * H]                        # r,z gates
        shared[f"bg{d}"] = f32(bfold.reshape(cfg.GS, 128).T)  # [128, GS]
        shared[f"bhn{d}"] = f32(b_hh[2 * H:].reshape(cfg.HK, 128).T)
    shared["identf"] = f32(np.eye(128))
    shared["identb"] = bf(np.eye(128))
    shared["bng"] = f32(np.asarray(inputs["bn_gamma"], np.float32)
                        .reshape(cfg.CHT, 128).T)
    shared["bnb"] = f32(np.asarray(inputs["bn_beta"], np.float32)
                        .reshape(cfg.CHT, 128).T)
    mlp_b = np.asarray(inputs["mlp_b"], np.float32)
    shared["mb65"] = f32(mlp_b.reshape(cfg.CHT, 128).T)
    shared["mwt"] = bf(np.asarray(inputs["mlp_W"], np.float32).T)
    lin_W = np.asarray(inputs["lin_W"], np.float32)
    lin_b = np.asarray(inputs["lin_b"], np.float32)
    shared["lwt"] = f32(lin_W.T)                             # [2H, C]
    shared["lb4"] = f32(np.broadcast_to(lin_b[None, :], (cfg.BC, C)))

    in_maps = []
    for core in range(cfg.n_cores):
        rows = x[core * cfg.BC:(core + 1) * cfg.BC]          # [BC, S]
        ids = np.zeros((cfg.BC, cfg.SEG), np.int64)
        ids[:, W - 1:W - 1 + S] = rows
        ids = ids.reshape(-1)                                # [TC]
        uids, inv = np.unique(ids, return_inverse=True)
        pt = np.zeros((cfg.TC, E), np.float16)
        pt[:len(uids)] = emb[uids].astype(np.float16)
        m = {k: v for k, v in shared.items()}
        m["ptab"] = pt
        m["ids"] = np.ascontiguousarray(inv.astype(np.int32)[:, None])
        mrow = mask[core * cfg.BC:(core + 1) * cfg.BC].reshape(-1)  # [NR]
        m["maskin"] = np.ascontiguousarray(
            np.broadcast_to(mrow[None, :], (128, cfg.NR)).astype(np.float16))
        in_maps.append(m)
    return in_maps


_CACHE = {}


def get_compiled(cfg: Cfg | None = None):
    key = "default" if cfg is None else id(cfg)
    if key not in _CACHE:
        _CACHE[key] = build(cfg or Cfg())
    return _CACHE[key]


def kernel(**inputs) -> np.ndarray:
    cfg = Cfg()
    nc = get_compiled(None)
    in_maps = prep_inputs(inputs, cfg)
    res = run_bass_kernel_spmd(nc, in_maps, core_ids=list(range(cfg.n_cores)))
    return np.concatenate([res.results[i]["out"] for i in range(cfg.n_cores)],
                          axis=0).astype(np.float32)
